# revision 8
# baseline (speedup 1.0000x reference)
"""Trainium2 Bass kernel for nn_DeconvLayer: double IIR deconv as a single FIR.

The reference applies a 16-tap IIR recurrence twice along seq (16384) for each
of 256 batch rows.  Both passes are linear, so the composition equals one
causal FIR convolution with the squared impulse response G2 = G * G, where
G is the impulse response of a single pass.  The largest characteristic root
here is ~0.904, so G2 truncated to 257 taps carries a relative tail of ~2e-10
— far below fp32 rounding.  This turns the sequential scan into fully
parallel banded matmuls.

Device mapping (8 cores = 2 batch halves x 4 seq quarters):
  - Host zero-pads + pre-transposes x into [s, j, b] 128-blocks (time-major),
    so tiles land in SBUF ready to act as matmul stationary operands — no
    on-device transposes at all.
  - out[b, t] (128x128 block i) accumulates in PSUM:
      e=1 (taps 1..255,  fp32): xT[i+1].T @ A1
      e=2 (taps 0..127,  fp32): xT[i+2].T @ A2
      e=0 (taps 129..256, bf16): xT[i].T @ A0   (|taps| <= 5.3e-6, so bf16
          rounding of x contributes < 4e-7 absolute — fp32-grade overall)
  - DVE evacuates PSUM to an SBUF staging buffer; grouped DMAs store y.
  - Input DMAs ride the HWDGE (sync) queues, output DMAs the SWDGE (gpsimd)
    queues so loads and stores don't share queue bandwidth.

Measured on trn2 (8 cores, in-kernel repeat loop, delta timing): ~29 us per
full 256x16384 problem (memory roofline ~12 us; jax reference scan is
orders of magnitude slower).
"""

import numpy as np

import concourse.bass as bass  # noqa: F401  (bass types used via tile/bacc)
import concourse.mybir as mybir
import concourse.tile as tile
from concourse import bacc
from concourse.bass_utils import run_bass_kernel_spmd

BATCH = 256
SEQ = 16384
F = 16
KT = 257          # FIR taps kept from G2
PAD = KT - 1      # 256 = 2 x 128 halo blocks
CORES = 8
SQ = 4            # seq split per batch half
CSEQ = SEQ // SQ  # 4096 output cols per core
NIN = CSEQ + PAD  # 4352 input cols per core
NBLK = NIN // 128   # 34 input blocks
NOUT = CSEQ // 128  # 32 output blocks
DMA_BLKS = 4        # input blocks per load DMA
OUT_GRP = 4         # output blocks per store DMA

_NC_CACHE = None
LAST_RESULTS = None  # BassKernelResults of the most recent run (for test.py)


def _impulse_response_sq(h: np.ndarray) -> np.ndarray:
    """First KT taps of the squared impulse response of v[n]=x[n]+h·v[n-1-j]."""
    g = np.zeros(KT, np.float64)
    g[0] = 1.0
    for n in range(1, KT):
        m = min(F, n)
        g[n] = h[:m] @ g[n - m:n][::-1]
    return np.convolve(g, g)[:KT]


def _filter_mats(g2: np.ndarray) -> np.ndarray:
    """A[s, e, t] = G2[256 - 128e + t - s] (0 outside [0, KT))."""
    s = np.arange(128)[:, None]
    t = np.arange(128)[None, :]
    a = np.zeros((128, 3, 128), np.float32)
    for e in range(3):
        k = PAD - 128 * e + t - s
        valid = (k >= 0) & (k < KT)
        a[:, e, :] = np.where(valid, g2[np.clip(k, 0, KT - 1)], 0.0)
    return a


def _build_nc(reps: int = 1):
    nc = bacc.Bacc("TRN2", target_bir_lowering=False, debug=False,
                   num_devices=CORES)
    xt_d = nc.dram_tensor("xt", [128, NBLK, 128], mybir.dt.float32,
                          kind="ExternalInput")
    am_d = nc.dram_tensor("amats", [128, 3, 128], mybir.dt.float32,
                          kind="ExternalInput")
    y_d = nc.dram_tensor("y", [128, NOUT, 128], mybir.dt.float32,
                         kind="ExternalOutput")

    with tile.TileContext(nc) as tc:
        with (
            tc.tile_pool(name="xin", bufs=2) as xin_pool,
            tc.tile_pool(name="am", bufs=1) as am_pool,
            tc.tile_pool(name="ysb", bufs=2) as out_pool,
            tc.tile_pool(name="acc", bufs=8, space="PSUM") as psum_pool,
        ):
            amt = am_pool.tile([128, 3, 128], mybir.dt.float32)
            nc.gpsimd.dma_start(amt[:], am_d[:])
            amt_bf = am_pool.tile([128, 128], mybir.dt.bfloat16,
                                  name="amt_bf", tag="amt_bf")
            nc.vector.tensor_copy(amt_bf[:], amt[:, 0, :])

            def body(_iv=None):
                xin = xin_pool.tile([128, NBLK, 128], mybir.dt.float32,
                                    name="xin_t", tag="xin_t")
                for c0 in range(0, NBLK, DMA_BLKS):
                    c1 = min(c0 + DMA_BLKS, NBLK)
                    nc.sync.dma_start(xin[:, c0:c1, :], xt_d[:, c0:c1, :])

                ysb = out_pool.tile([128, NOUT, 128], mybir.dt.float32,
                                    name="ysb_t", tag="ysb_t")
                acc = {}
                xbf = {}
                for j in range(NBLK):
                    if j < NOUT:
                        xbf[j] = out_pool.tile([128, 128], mybir.dt.bfloat16,
                                               name=f"xbf{j}", tag="xbf",
                                               bufs=3)
                        nc.gpsimd.tensor_copy(xbf[j][:], xin[:, j, :])
                    for e in (0, 1, 2):
                        i = j - e
                        if not (0 <= i < NOUT):
                            continue
                        if e == 0:
                            acc[i] = psum_pool.tile(
                                [128, 128], mybir.dt.float32,
                                name=f"acc{i}", tag="acc")
                            nc.tensor.matmul(acc[i][:], xbf[j][:], amt_bf[:],
                                             start=True, stop=False)
                        else:
                            nc.tensor.matmul(acc[i][:], xin[:, j, :],
                                             amt[:, e, :],
                                             start=False, stop=(e == 2))
                    i = j - 2
                    if 0 <= i < NOUT:
                        nc.vector.tensor_copy(ysb[:, i, :], acc[i][:])
                        del acc[i]
                        if (i + 1) % OUT_GRP == 0:
                            g0 = i + 1 - OUT_GRP
                            nc.gpsimd.dma_start(y_d[:, g0:i + 1, :],
                                                ysb[:, g0:i + 1, :])

            if reps == 1:
                body()
            else:
                # bench-only loop; PE body exceeds one IRAM block, so arm
                # the branch prefetcher to avoid a ~4us I$-miss per back-edge
                with tc.For_i(0, reps, 1,
                              hint_engines=(mybir.EngineType.PE,)) as iv:
                    body(iv)
    nc.compile()
    return nc


def _get_nc(reps: int = 1):
    global _NC_CACHE
    if _NC_CACHE is None:
        _NC_CACHE = {}
    if reps not in _NC_CACHE:
        _NC_CACHE[reps] = _build_nc(reps)
    return _NC_CACHE[reps]


def kernel(inputs: np.ndarray, kernel: np.ndarray,
           _reps: int = 1) -> np.ndarray:
    global LAST_RESULTS
    x = np.asarray(inputs, np.float32)
    h = np.asarray(kernel, np.float64)[0]
    assert x.shape == (BATCH, SEQ) and h.shape == (F,)

    g2 = _impulse_response_sq(h)
    amats = _filter_mats(g2)

    # Xpad[:, c] = x~[:, c - PAD] where x~ is x with cols < 16 zeroed
    # (the reference zeroes v[0:16] and never reads x[:, 0:16]).
    xpad = np.zeros((BATCH, PAD + SEQ), np.float32)
    xpad[:, PAD + 16:] = x[:, 16:]

    in_maps = []
    for c in range(CORES):
        bh, q = divmod(c, SQ)
        sl = xpad[bh * 128:(bh + 1) * 128, q * CSEQ: q * CSEQ + NIN]
        # [b, c'] -> [s, j, b] time-major blocks
        xt = np.ascontiguousarray(
            sl.T.reshape(NBLK, 128, 128).transpose(1, 0, 2))
        in_maps.append({"xt": xt, "amats": amats})

    nc = _get_nc(_reps)
    LAST_RESULTS = run_bass_kernel_spmd(nc, in_maps,
                                        core_ids=list(range(CORES)))

    y = np.empty((BATCH, SEQ), np.float32)
    for c in range(CORES):
        bh, q = divmod(c, SQ)
        y[bh * 128:(bh + 1) * 128, q * CSEQ:(q + 1) * CSEQ] = \
            LAST_RESULTS.results[c]["y"].reshape(128, CSEQ)
    return y


# revision 9
# speedup vs baseline: 1.1241x; 1.1241x over previous
"""Trainium2 Bass kernel for nn_DeconvLayer: double IIR deconv as a single FIR.

The reference applies a 16-tap IIR recurrence twice along seq (16384) for each
of 256 batch rows.  Both passes are linear, so the composition equals one
causal FIR convolution with the squared impulse response G2 = G * G, where
G is the impulse response of a single pass.  The largest characteristic root
here is ~0.904, so G2 truncated to 257 taps carries a relative tail of ~2e-10
— far below fp32 rounding.  This turns the sequential scan into fully
parallel banded matmuls.

Device mapping (8 cores = 2 batch halves x 4 seq quarters):
  - Host zero-pads + pre-transposes x into [s, j, b] 128-blocks (time-major),
    so tiles land in SBUF ready to act as matmul stationary operands — no
    on-device transposes at all.
  - out[b, t] (128x128 block i) accumulates in PSUM:
      e=1 (taps 1..255,  fp32): xT[i+1].T @ A1
      e=2 (taps 0..127,  fp32): xT[i+2].T @ A2
      e=0 (taps 129..256, bf16): xT[i].T @ A0   (|taps| <= 5.3e-6, so bf16
          rounding of x contributes < 4e-7 absolute — fp32-grade overall)
  - DVE evacuates PSUM to an SBUF staging buffer; grouped DMAs store y.
  - Input DMAs ride the HWDGE (sync) queues, output DMAs the SWDGE (gpsimd)
    queues so loads and stores don't share queue bandwidth.

Measured on trn2 (8 cores, in-kernel repeat loop, delta timing): ~29 us per
full 256x16384 problem (memory roofline ~12 us; jax reference scan is
orders of magnitude slower).
"""

import numpy as np

import concourse.bass as bass  # noqa: F401  (bass types used via tile/bacc)
import concourse.mybir as mybir
import concourse.tile as tile
from concourse import bacc
from concourse.bass_utils import run_bass_kernel_spmd

BATCH = 256
SEQ = 16384
F = 16
KT = 257          # FIR taps kept from G2
PAD = KT - 1      # 256 = 2 x 128 halo blocks
CORES = 8
SQ = 4            # seq split per batch half
CSEQ = SEQ // SQ  # 4096 output cols per core
NIN = CSEQ + PAD  # 4352 input cols per core
NBLK = NIN // 128   # 34 input blocks
NOUT = CSEQ // 128  # 32 output blocks
DMA_BLKS = 4        # input blocks per load DMA
OUT_GRP = 4         # output blocks per store DMA

_NC_CACHE = None
LAST_RESULTS = None  # BassKernelResults of the most recent run (for test.py)


def _impulse_response_sq(h: np.ndarray) -> np.ndarray:
    """First KT taps of the squared impulse response of v[n]=x[n]+h·v[n-1-j]."""
    g = np.zeros(KT, np.float64)
    g[0] = 1.0
    for n in range(1, KT):
        m = min(F, n)
        g[n] = h[:m] @ g[n - m:n][::-1]
    return np.convolve(g, g)[:KT]


def _filter_mats(g2: np.ndarray) -> np.ndarray:
    """A[s, e, t] = G2[256 - 128e + t - s] (0 outside [0, KT))."""
    s = np.arange(128)[:, None]
    t = np.arange(128)[None, :]
    a = np.zeros((128, 3, 128), np.float32)
    for e in range(3):
        k = PAD - 128 * e + t - s
        valid = (k >= 0) & (k < KT)
        a[:, e, :] = np.where(valid, g2[np.clip(k, 0, KT - 1)], 0.0)
    return a


def _build_nc(reps: int = 1):
    nc = bacc.Bacc("TRN2", target_bir_lowering=False, debug=False,
                   num_devices=CORES)
    xt_d = nc.dram_tensor("xt", [128, NBLK, 128], mybir.dt.float32,
                          kind="ExternalInput")
    am_d = nc.dram_tensor("amats", [128, 3, 128], mybir.dt.float32,
                          kind="ExternalInput")
    y_d = nc.dram_tensor("y", [128, NOUT, 128], mybir.dt.float32,
                         kind="ExternalOutput")

    with tile.TileContext(nc) as tc:
        with (
            tc.tile_pool(name="xin", bufs=2) as xin_pool,
            tc.tile_pool(name="am", bufs=1) as am_pool,
            tc.tile_pool(name="ysb", bufs=2) as out_pool,
            tc.tile_pool(name="acc", bufs=8, space="PSUM") as psum_pool,
        ):
            amt = am_pool.tile([128, 3, 128], mybir.dt.float32)
            nc.gpsimd.dma_start(amt[:], am_d[:])
            amt_bf = am_pool.tile([128, 128], mybir.dt.bfloat16,
                                  name="amt_bf", tag="amt_bf")
            nc.vector.tensor_copy(amt_bf[:], amt[:, 0, :])

            def body(_iv=None):
                xin = xin_pool.tile([128, NBLK, 128], mybir.dt.float32,
                                    name="xin_t", tag="xin_t")
                for c0 in range(0, NBLK, DMA_BLKS):
                    c1 = min(c0 + DMA_BLKS, NBLK)
                    nc.sync.dma_start(xin[:, c0:c1, :], xt_d[:, c0:c1, :])

                ysb = out_pool.tile([128, NOUT, 128], mybir.dt.float32,
                                    name="ysb_t", tag="ysb_t")
                acc = {}
                xbf = {}
                for j in range(NBLK):
                    if j < NOUT:
                        xbf[j] = out_pool.tile([128, 128], mybir.dt.bfloat16,
                                               name=f"xbf{j}", tag="xbf",
                                               bufs=3)
                        nc.vector.tensor_copy(xbf[j][:], xin[:, j, :])
                    for e in (0, 1, 2):
                        i = j - e
                        if not (0 <= i < NOUT):
                            continue
                        if e == 0:
                            acc[i] = psum_pool.tile(
                                [128, 128], mybir.dt.float32,
                                name=f"acc{i}", tag="acc")
                            nc.tensor.matmul(acc[i][:], xbf[j][:], amt_bf[:],
                                             start=True, stop=False)
                        else:
                            nc.tensor.matmul(acc[i][:], xin[:, j, :],
                                             amt[:, e, :],
                                             start=False, stop=(e == 2))
                    i = j - 2
                    if 0 <= i < NOUT:
                        nc.vector.tensor_copy(ysb[:, i, :], acc[i][:])
                        del acc[i]
                        if (i + 1) % OUT_GRP == 0:
                            g0 = i + 1 - OUT_GRP
                            nc.gpsimd.dma_start(y_d[:, g0:i + 1, :],
                                                ysb[:, g0:i + 1, :])

            if reps == 1:
                body()
            else:
                # bench-only loop; PE body exceeds one IRAM block, so arm
                # the branch prefetcher to avoid a ~4us I$-miss per back-edge
                with tc.For_i(0, reps, 1,
                              hint_engines=(mybir.EngineType.PE,)) as iv:
                    body(iv)
    nc.compile()
    return nc


def _get_nc(reps: int = 1):
    global _NC_CACHE
    if _NC_CACHE is None:
        _NC_CACHE = {}
    if reps not in _NC_CACHE:
        _NC_CACHE[reps] = _build_nc(reps)
    return _NC_CACHE[reps]


def kernel(inputs: np.ndarray, kernel: np.ndarray,
           _reps: int = 1) -> np.ndarray:
    global LAST_RESULTS
    x = np.asarray(inputs, np.float32)
    h = np.asarray(kernel, np.float64)[0]
    assert x.shape == (BATCH, SEQ) and h.shape == (F,)

    g2 = _impulse_response_sq(h)
    amats = _filter_mats(g2)

    # Xpad[:, c] = x~[:, c - PAD] where x~ is x with cols < 16 zeroed
    # (the reference zeroes v[0:16] and never reads x[:, 0:16]).
    xpad = np.zeros((BATCH, PAD + SEQ), np.float32)
    xpad[:, PAD + 16:] = x[:, 16:]

    in_maps = []
    for c in range(CORES):
        bh, q = divmod(c, SQ)
        sl = xpad[bh * 128:(bh + 1) * 128, q * CSEQ: q * CSEQ + NIN]
        # [b, c'] -> [s, j, b] time-major blocks
        xt = np.ascontiguousarray(
            sl.T.reshape(NBLK, 128, 128).transpose(1, 0, 2))
        in_maps.append({"xt": xt, "amats": amats})

    nc = _get_nc(_reps)
    LAST_RESULTS = run_bass_kernel_spmd(nc, in_maps,
                                        core_ids=list(range(CORES)))

    y = np.empty((BATCH, SEQ), np.float32)
    for c in range(CORES):
        bh, q = divmod(c, SQ)
        y[bh * 128:(bh + 1) * 128, q * CSEQ:(q + 1) * CSEQ] = \
            LAST_RESULTS.results[c]["y"].reshape(128, CSEQ)
    return y
